# revision 19
# baseline (speedup 1.0000x reference)
"""Trainium2 Bass kernel for InterModalRankingLoss (B=4096, D=1024, k=1024).

Strategy (8 NeuronCores, SPMD — identical program, per-core data):
  core c owns rows [512c, 512c+512) of dist (loss_xy) and the same slice of
  dist.T (loss_yx). Each [512, 4096] block: 8x accumulated bf16 matmuls per
  [128,512] PSUM tile; c_j column term added via DVE scalar_tensor_tensor from
  a host-replicated broadcast tile; d = sqrt(-2*acc + r_i) fused on ScalarE
  (per-partition bias) writing fp16 d with a free fp32 row-sum (accum_out).

  Top-k=1024 per row without sorting: the loss contribution per row equals
  G + k*max(a - tau, 0) where a = margin + pos_i, tau ~= k-th smallest
  distance (found with 2 counting probes + secant step, all on-chip), and
  G = sum(relu(theta - d)) with theta = min(tau, a) via ScalarE Relu+accum.
  The k*(a-tau) correction makes the result insensitive to tau error to
  first order. Host sums per-row results (fp64) — no device collectives.

Self-contained: hardcodes shapes; only imports concourse from /opt.
"""
import sys
import numpy as np
import ml_dtypes

sys.path.insert(0, "/opt/trn_rl_repo")

import concourse.bass as bass
import concourse.mybir as mybir
import concourse.tile as tile
from concourse import bacc
from concourse.bass_utils import run_bass_kernel_spmd

P = 128
B = 4096
D = 1024
K_SEL = 1024
RB = 512            # rows per core-block
NRT = RB // P       # 4 row-tiles
NJT = B // 512      # 8 column tiles of 512
NKC = D // P        # 8 contraction chunks
MARGIN = 0.5
EPS = 1e-6
DIAG_VAL = 60000.0
Z0 = -0.6745
ZA, ZB = Z0 - 0.10, Z0 + 0.10

F32 = mybir.dt.float32
F16 = mybir.dt.float16
BF16 = mybir.dt.bfloat16
ALU = mybir.AluOpType
ACTF = mybir.ActivationFunctionType

# host param columns (hp tensor, per row)
HP_BIAS, HP_POS, HP_A, HP_W0, HP_W1, HP_DNEG, HP_LOOFF, HP_HIOFF = range(8)
# output columns
OC_G, OC_TAU, OC_MU, OC_CA, OC_CB, OC_SUMD = range(6)

_NC_CACHE = {}


def build_nc():
    nc = bacc.Bacc(None)
    lhs_d = [nc.declare_dram_parameter(f"lhs{b}", [D, RB], BF16, isOutput=False) for b in range(2)]
    rhs_d = [nc.declare_dram_parameter(f"rhs{b}", [D, B], BF16, isOutput=False) for b in range(2)]
    cbc_d = [nc.declare_dram_parameter(f"cbc{b}", [P, B], F32, isOutput=False) for b in range(2)]
    hp_d = [nc.declare_dram_parameter(f"hp{b}", [P, NRT * 8], F32, isOutput=False) for b in range(2)]
    id_d = nc.declare_dram_parameter("ident", [P, P], F16, isOutput=False)
    outp = nc.declare_dram_parameter("outp", [2, NRT, P, 8], F32, isOutput=True)

    with tile.TileContext(nc) as tc:
        with (
            tc.tile_pool(name="dpool", bufs=2 * NRT) as dpool,
            tc.tile_pool(name="rhsp", bufs=3) as rhsp,
            tc.tile_pool(name="lhsp", bufs=2) as lhsp,
            tc.tile_pool(name="cbcp", bufs=2) as cbcp,
            tc.tile_pool(name="psum", bufs=6, space="PSUM") as psp,
            tc.tile_pool(name="tmpp", bufs=4) as tmpp,
            tc.tile_pool(name="junkp", bufs=3) as junkp,
            tc.tile_pool(name="smallp", bufs=2) as smallp,
            tc.tile_pool(name="outs", bufs=4) as outsp,
        ):
            id_sb = smallp.tile([P, P], F16, tag="ident")
            nc.sync.dma_start(out=id_sb[:], in_=id_d[:])

            for blk in range(2):
                lhs_v = lhs_d[blk][:].rearrange("(kc p) m -> p kc m", p=P)
                rhs_v = rhs_d[blk][:].rearrange("(kc p) n -> p kc n", p=P)

                lhs_sb = lhsp.tile([P, NKC, RB], BF16, tag="lhs")
                nc.sync.dma_start(out=lhs_sb[:], in_=lhs_v)
                cbc_sb = cbcp.tile([P, B], F32, tag="cbc")
                for jt in range(NJT):
                    nc.sync.dma_start(
                        out=cbc_sb[:, jt * 512:(jt + 1) * 512],
                        in_=cbc_d[blk][:, jt * 512:(jt + 1) * 512],
                    )
                hp_sb = smallp.tile([P, NRT * 8], F32, tag="hp")
                nc.sync.dma_start(out=hp_sb[:], in_=hp_d[blk][:])
                stats = smallp.tile([P, NRT * NJT], F32, tag="stats")

                d_tiles = []
                for rt in range(NRT):
                    d_tiles.append(dpool.tile([P, B], F16, tag="dt", name=f"d_b{blk}_rt{rt}"))

                def hp(rt, col):
                    return hp_sb[:, rt * 8 + col: rt * 8 + col + 1]

                for jt in range(NJT):
                    rhs_sb = rhsp.tile([P, NKC, 512], BF16, tag="rhs")
                    nc.sync.dma_start(out=rhs_sb[:], in_=rhs_v[:, :, jt * 512:(jt + 1) * 512])
                    for rt in range(NRT):
                        ps = psp.tile([P, 512], F32, tag="ps")
                        for kc in range(NKC):
                            nc.tensor.matmul(
                                ps[:],
                                lhs_sb[:, kc, rt * P:(rt + 1) * P],
                                rhs_sb[:, kc, :],
                                start=(kc == 0),
                                stop=(kc == NKC - 1),
                            )
                        tmp = tmpp.tile([P, 512], F32, tag="tmp")
                        # tmp = ps - c_j/2  (cbc holds c/2)
                        nc.vector.tensor_sub(
                            tmp[:], ps[:], cbc_sb[:, jt * 512:(jt + 1) * 512],
                        )
                        # d = sqrt(-2*tmp + r_i), fp16 out, fp32 row-sum accum
                        nc.scalar.activation(
                            out=d_tiles[rt][:, jt * 512:(jt + 1) * 512],
                            in_=tmp[:], func=ACTF.Sqrt,
                            bias=hp(rt, HP_BIAS), scale=-2.0,
                            accum_out=stats[:, rt * NJT + jt: rt * NJT + jt + 1],
                        )

                for rt in range(NRT):
                    dt_ = d_tiles[rt]
                    # diagonal -> DIAG_VAL (cols rt*P..rt*P+P in rotated layout);
                    # id_sb is the identity pre-scaled by DIAG_VAL on the host
                    nc.vector.tensor_max(
                        dt_[:, rt * P:(rt + 1) * P], id_sb[:],
                        dt_[:, rt * P:(rt + 1) * P],
                    )
                    ost = outsp.tile([P, 8], F32, tag="ost")
                    sumd = ost[:, OC_SUMD:OC_SUMD + 1]
                    nc.vector.tensor_reduce(
                        out=sumd, in_=stats[:, rt * NJT:(rt + 1) * NJT],
                        axis=mybir.AxisListType.X, op=ALU.add,
                    )
                    mu = ost[:, OC_MU:OC_MU + 1]
                    # mu = (sumd - pos) / (B-1)
                    nc.vector.tensor_scalar(
                        out=mu, in0=sumd, scalar1=hp(rt, HP_POS),
                        scalar2=1.0 / (B - 1), op0=ALU.subtract, op1=ALU.mult,
                    )
                    taua = smallp.tile([P, 1], F32, tag="taua")
                    taub = smallp.tile([P, 1], F32, tag="taub")
                    nc.vector.tensor_scalar_add(taua[:], mu, hp(rt, HP_W0))
                    nc.vector.tensor_scalar_add(taub[:], mu, hp(rt, HP_W1))
                    ca = ost[:, OC_CA:OC_CA + 1]
                    cb = ost[:, OC_CB:OC_CB + 1]
                    junk1 = junkp.tile([P, B], F16, tag="junk")
                    nc.vector.tensor_scalar(
                        out=junk1[:], in0=dt_[:], scalar1=taua[:], scalar2=0.0,
                        op0=ALU.is_le, op1=ALU.add, accum_out=ca,
                    )
                    junk2 = junkp.tile([P, B], F16, tag="junk")
                    nc.vector.tensor_scalar(
                        out=junk2[:], in0=dt_[:], scalar1=taub[:], scalar2=0.0,
                        op0=ALU.is_le, op1=ALU.add, accum_out=cb,
                    )
                    dden = smallp.tile([P, 1], F32, tag="dden")
                    nc.vector.tensor_scalar(
                        out=dden[:], in0=cb, scalar1=ca, scalar2=0.5,
                        op0=ALU.subtract, op1=ALU.max,
                    )
                    rcp = smallp.tile([P, 1], F32, tag="rcp")
                    nc.vector.reciprocal(rcp[:], dden[:])
                    dnum = smallp.tile([P, 1], F32, tag="dnum")
                    nc.vector.tensor_scalar_sub(dnum[:], ca, float(K_SEL))
                    m1 = smallp.tile([P, 1], F32, tag="m1")
                    nc.vector.tensor_scalar_mul(m1[:], dnum[:], rcp[:])
                    m2 = smallp.tile([P, 1], F32, tag="m2")
                    nc.vector.tensor_scalar_mul(m2[:], m1[:], hp(rt, HP_DNEG))
                    taur = smallp.tile([P, 1], F32, tag="taur")
                    nc.vector.tensor_scalar_add(taur[:], taua[:], m2[:])
                    # clamp to [mu+lo_off, mu+hi_off]
                    bnd = smallp.tile([P, 1], F32, tag="bnd")
                    nc.vector.tensor_scalar_add(bnd[:], mu, hp(rt, HP_HIOFF))
                    nc.vector.tensor_scalar_min(taur[:], taur[:], bnd[:])
                    bnd2 = smallp.tile([P, 1], F32, tag="bnd2")
                    nc.vector.tensor_scalar_add(bnd2[:], mu, hp(rt, HP_LOOFF))
                    tau = ost[:, OC_TAU:OC_TAU + 1]
                    nc.vector.tensor_scalar_max(tau, taur[:], bnd2[:])
                    theta = smallp.tile([P, 1], F32, tag="theta")
                    nc.vector.tensor_scalar_min(theta[:], tau, hp(rt, HP_A))
                    junk3 = junkp.tile([P, B], F16, tag="junk")
                    nc.scalar.activation(
                        out=junk3[:], in_=dt_[:], func=ACTF.Relu,
                        bias=theta[:], scale=-1.0,
                        accum_out=ost[:, OC_G:OC_G + 1],
                    )
                    nc.sync.dma_start(out=outp[blk, rt, :, 0:6], in_=ost[:, 0:6])
    nc.compile()
    return nc


def _host_prep(x, y):
    x64, y64 = x.astype(np.float64), y.astype(np.float64)
    r = (np.sum(x64 * x64, 1) + 2 * EPS * np.sum(x64, 1) + D * EPS * EPS).astype(np.float32)
    c = (np.sum(y64 * y64, 1) - 2 * EPS * np.sum(y64, 1)).astype(np.float32)
    pos_sq = r + c - 2.0 * np.einsum("ij,ij->i", x64, y64).astype(np.float32)
    pos = np.sqrt(np.maximum(pos_sq, 0)).astype(np.float32)
    a_vec = (MARGIN + pos).astype(np.float32)

    var_c = float(np.var(c)); lam_y = float(np.mean(np.var(y64, axis=0)))
    mean_c = float(np.mean(c))
    sig_A = (np.sqrt(var_c + 4 * np.sum(x64 * x64, 1) * lam_y)
             / (2 * np.sqrt(np.maximum(r + mean_c, 1e-6)))).astype(np.float32)
    var_r = float(np.var(r)); lam_x = float(np.mean(np.var(x64, axis=0)))
    mean_r = float(np.mean(r))
    sig_B = (np.sqrt(var_r + 4 * np.sum(y64 * y64, 1) * lam_x)
             / (2 * np.sqrt(np.maximum(c + mean_r, 1e-6)))).astype(np.float32)
    return r, c, pos, a_vec, sig_A, sig_B


def _hp_pack(bias_rows, pos_rows, a_rows, sig_rows):
    hp = np.zeros((RB, 8), np.float32)
    hp[:, HP_BIAS] = bias_rows
    hp[:, HP_POS] = pos_rows
    hp[:, HP_A] = a_rows
    hp[:, HP_W0] = ZA * sig_rows
    hp[:, HP_W1] = ZB * sig_rows
    hp[:, HP_DNEG] = (ZA - ZB) * sig_rows
    hp[:, HP_LOOFF] = (Z0 - 1.0) * sig_rows
    hp[:, HP_HIOFF] = (Z0 + 1.0) * sig_rows
    # on-chip layout: [P, NRT*8] with row rt*P+p at [p, rt*8:rt*8+8]
    return np.ascontiguousarray(hp.reshape(NRT, P, 8).transpose(1, 0, 2).reshape(P, NRT * 8))


def kernel(x_embed, y_embed):
    x = np.ascontiguousarray(np.asarray(x_embed, np.float32))
    y = np.ascontiguousarray(np.asarray(y_embed, np.float32))
    r, c, pos, a_vec, sig_A, sig_B = _host_prep(x, y)

    xT16 = np.ascontiguousarray(x.astype(ml_dtypes.bfloat16).T)
    yT16 = np.ascontiguousarray(y.astype(ml_dtypes.bfloat16).T)
    ident = (DIAG_VAL * np.eye(P)).astype(np.float16)

    in_maps = []
    for core in range(8):
        s = core * RB
        rot = np.concatenate([np.arange(s, B), np.arange(0, s)])
        in_maps.append({
            "lhs0": np.ascontiguousarray(xT16[:, s:s + RB]),
            "rhs0": np.ascontiguousarray(yT16[:, rot]),
            "cbc0": np.ascontiguousarray(np.broadcast_to(0.5 * c[rot], (P, B))),
            "hp0": _hp_pack(r[s:s + RB], pos[s:s + RB], a_vec[s:s + RB], sig_A[s:s + RB]),
            "lhs1": np.ascontiguousarray(yT16[:, s:s + RB]),
            "rhs1": np.ascontiguousarray(xT16[:, rot]),
            "cbc1": np.ascontiguousarray(np.broadcast_to(0.5 * r[rot], (P, B))),
            "hp1": _hp_pack(c[s:s + RB], pos[s:s + RB], a_vec[s:s + RB], sig_B[s:s + RB]),
            "ident": ident,
        })

    if "nc" not in _NC_CACHE:
        _NC_CACHE["nc"] = build_nc()
    nc = _NC_CACHE["nc"]
    res = run_bass_kernel_spmd(nc, in_maps, list(range(8)),
                               trace=bool(_NC_CACHE.get("trace")))
    _NC_CACHE["last_res"] = res

    tot = np.zeros(2, np.float64)
    for core in range(8):
        s = core * RB
        a_rows = a_vec[s:s + RB].astype(np.float64)
        o = np.asarray(res.results[core]["outp"], np.float64)  # [2, NRT, P, 8]
        for blk in range(2):
            G = o[blk, :, :, OC_G].reshape(RB)
            tau = o[blk, :, :, OC_TAU].reshape(RB)
            contrib = G + K_SEL * np.maximum(a_rows - tau, 0.0)
            tot[blk] += contrib.sum()
    denom = float(B) * float(K_SEL)
    return (np.float32(tot[0] / denom), np.float32(tot[1] / denom))


if __name__ == "__main__":
    nc = build_nc()
    print("build OK, instructions:",
          sum(len(bb.instructions) for bb in nc.main_func.blocks))
